# revision 17
# baseline (speedup 1.0000x reference)
"""Trainium2 Bass kernel for CharacterNet segment-mean + FC (segment_reduce).

Reference computation (per batch row b of 32):
  x = all_encoder_layers[layer_index][b]          # (512, 768)
  for t in 0..255: mean_t = mean(x[token_map[b,t]:token_map[b,t+1]])
  ote[b*256+t] = mean_t                           # (8192, 768) output 2
  rep = ote @ fc_w.T + fc_b                       # (8192, 768) output 1

Strategy: data-parallel over batch across 8 NeuronCores (4 rows/core).
The segment mean is a matmul with a one-hot-per-row selection matrix
SelT (512, 256), SelT[s, t] = (seg(s)==t) / count(seg(s)), built on
device from two tiny per-position index vectors with a single
tensor_scalar op per s-chunk.  Stage 1 computes meanT = x.T @ SelT
(H on partitions) so stage 2 (the FC) consumes it directly as the
stationary operand.

PE-cycle economy (the kernel is Tensor-engine bound):
 - All DRAM I/O is bf16 (inputs converted on host, outputs upcast on
   host); fp32 accumulation in PSUM keeps error ~4e-3 vs the 2e-2 gate.
 - Sparse stage 1: segments are contiguous s-ranges, so the 128-row
   s-chunk ks only touches output columns [lo_ks, hi_ks] (~100 of 256,
   union over the 8 SPMD cores, derived from token_map at build time).
   Matmuls cover only the active band: fresh columns start=True,
   columns shared with earlier chunks accumulate with start=False
   (seg jumps by <=1 per position, so bands are gap-free).
 - ote is written to DRAM transposed (oteT = meanT layout) straight
   from the stage-1 SBUF tiles - no PE transposes, no extra evictions;
   the host transposes for free during reassembly.
 - The K=1 bias matmuls are skipped when fc_b == 0.
"""

import numpy as np
import ml_dtypes

import concourse.bass as bass
import concourse.bacc as bacc
import concourse.mybir as mybir
import concourse.tile as tile
from concourse.bass_utils import run_bass_kernel_spmd  # noqa: F401 (contract)

N_CORES = 8
B, S, H, T = 32, 512, 768, 256
B_LOC = B // N_CORES          # 4 batch rows per core
NS = S // 128                 # 4 s-chunks per row
NJ = B_LOC * NS               # 16 (128,768) x chunks per core
NH = H // 128                 # 6 h-chunks
NB2 = 384                     # stage-2 N tile (two per 768)

F32 = mybir.dt.float32
BF16 = mybir.dt.bfloat16
NP_BF16 = ml_dtypes.bfloat16

# tunables: engine for each PSUM-evict copy class, DMA trigger engines,
# psum pool sizes, DMA emission order
OPT = {
    "m_copy": "vector",      # meanT psum->sbuf: vector | scalar
    "rep_copy": "scalar",    # stage2 psum->sbuf: vector | scalar
    "p1": 4, "p2": 4,
    "in_dma": "sync",        # input DMA trigger engine
    "ote_dma": "gpsimd",     # oteT DMA trigger engine (Pool is idle)
    "rep_dma": "scalar",     # rep DMA trigger engine
    "w_after": 3,            # emit fc_w DMAs after this many x2 DMAs
    "x_split_first": True,   # first x2 pair as two DMAs (earlier PE start)
    "unroll": 2,             # emit_rep per For_i iteration (loop mode)
    "dbl": True,             # bufs=2 on x/sel/m/w pools (pairs with unroll=2)
    "s1_style": "split",     # split (fresh/overlap matmuls) | memset
    "s1_memset_eng": "vector",
    "s2_interleave": True,   # reuse stage-2 stationary across both nh halves
}

# dense fallback ranges: every chunk covers all T columns
DENSE_RANGES = tuple(
    tuple((0, T - 1) for _ in range(NS)) for _ in range(B_LOC))


def _copy(nc, engine, dst, src_):
    if engine == "scalar":
        nc.scalar.copy(dst, src_)
    else:
        nc.vector.tensor_copy(dst, src_)


def build_kernel(reps: int = 1, loop: bool = False,
                 bias_mm: bool = False,
                 ranges=DENSE_RANGES) -> bass.Bass:
    nc = bacc.Bacc("TRN2", target_bir_lowering=False, debug=False,
                   num_devices=N_CORES)

    x_d = nc.dram_tensor("x", (NJ * 128, H), BF16, kind="ExternalInput")
    # packed aux: cols 0..15 = seg, 16..31 = inv  (128, 32)
    # (fp32: tensor_scalar is_equal requires fp32 scalar operands)
    aux_d = nc.dram_tensor("selaux", (128, 2 * NJ), F32, kind="ExternalInput")
    fcw_d = nc.dram_tensor("fcwT", (H, H), BF16, kind="ExternalInput")
    # packed bias row: [0:H]=fc_b, [H:H+128]=ones
    bias_d = nc.dram_tensor("biasaux", (1, H + 128), BF16, kind="ExternalInput")
    rep_d = nc.dram_tensor("rep", (B_LOC * T, H), BF16, kind="ExternalOutput")
    # ote in meanT orientation; host transposes during reassembly
    otT_d = nc.dram_tensor("otT", (H, B_LOC * T), BF16, kind="ExternalOutput")

    # paired-row-chunk views for big DMAs: [j0][p, q, h] = t[(2*j0+q)*128+p, h]
    x_v = x_d.rearrange("(a q p) h -> a p q h", q=2, p=128)
    rep_v = rep_d.rearrange("(a q p) h -> a p q h", q=2, p=128)

    dbl = 2 if OPT["dbl"] else 1
    with tile.TileContext(nc) as tc:
        with (
            tc.tile_pool(name="const", bufs=1) as cpool,
            tc.tile_pool(name="xp", bufs=dbl) as xpool,
            tc.tile_pool(name="selp", bufs=dbl) as selpool,
            tc.tile_pool(name="mp", bufs=dbl) as mpool,
            tc.tile_pool(name="wp", bufs=dbl) as wpool,
            tc.tile_pool(name="ob", bufs=2) as opool,
            tc.tile_pool(name="p1", bufs=OPT["p1"], space="PSUM") as p1pool,
            tc.tile_pool(name="p2", bufs=OPT["p2"], space="PSUM") as p2pool,
        ):
            # one-time constants
            iota_t = cpool.tile([128, T], BF16, tag="iota")
            nc.gpsimd.iota(iota_t[:], pattern=[[1, T]], base=0,
                           channel_multiplier=0,
                           allow_small_or_imprecise_dtypes=True)

            def emit_rep():
                aux_sb = cpool.tile([128, 2 * NJ], F32, tag="aux")
                bias_sb = cpool.tile([1, H + 128], BF16, tag="bias")
                idma = getattr(nc, OPT["in_dma"])
                idma.dma_start(aux_sb[:], aux_d[:])
                idma.dma_start(bias_sb[:], bias_d[:])
                fcb_sb = bias_sb[:1, 0:H]
                ones = bias_sb[:1, H:H + 128]

                w_sb, x2_sb = [], []

                def emit_w():
                    for k in range(NH):
                        w = wpool.tile([128, H], BF16, tag=f"w{k}")
                        idma.dma_start(w[:], fcw_d[k * 128:(k + 1) * 128, :])
                        w_sb.append(w)

                for j0 in range(NJ // 2):
                    if j0 == OPT["w_after"]:
                        emit_w()
                    x2 = xpool.tile([128, 2 * H], BF16, tag=f"x{j0}")
                    if j0 == 0 and OPT["x_split_first"]:
                        for q in range(2):
                            idma.dma_start(
                                x2[:, q * H:(q + 1) * H],
                                x_d[q * 128:(q + 1) * 128, :])
                    else:
                        idma.dma_start(
                            x2[:].rearrange("p (q h) -> p q h", q=2),
                            x_v[j0])
                    x2_sb.append(x2)
                if OPT["w_after"] >= NJ // 2:
                    emit_w()

                def x_chunk(j, mh):
                    # (128,128) stationary slice of wp-token chunk j, h-chunk mh
                    q, j0 = j % 2, j // 2
                    o = q * H + mh * 128
                    return x2_sb[j0][:, o:o + 128]

                sel_sb = {}
                for b in range(B_LOC):
                    for ks in range(NS):
                        lo, hi = ranges[b][ks]
                        j = b * NS + ks
                        sel = selpool.tile([128, hi - lo + 1], BF16,
                                           tag=f"s{j}")
                        # Sel^T band: (s==seg member of segment t) / count
                        nc.vector.tensor_scalar(
                            sel[:], iota_t[:, lo:hi + 1],
                            aux_sb[:, j:j + 1], aux_sb[:, NJ + j:NJ + j + 1],
                            op0=mybir.AluOpType.is_equal,
                            op1=mybir.AluOpType.mult)
                        sel_sb[j] = sel

                odma_o = getattr(nc, OPT["ote_dma"])
                odma_r = getattr(nc, OPT["rep_dma"])
                # meanT rows for all 4 b side by side: one wide oteT DMA
                # per h-chunk (2KB per-partition lines)
                m_all = []
                for mh in range(NH):
                    m_row = mpool.tile([128, B_LOC * T], BF16, tag=f"m{mh}")
                    m_all.append(m_row)
                for b in range(B_LOC):
                    # stage 1: meanT[b] (768, 256) = x[b].T @ SelT[b]
                    # split each chunk's band into already-covered columns
                    # (accumulate) and fresh columns (start=True)
                    mb = []
                    for mh in range(NH):
                        m = m_all[mh][:, b * T:(b + 1) * T]
                        ps = p1pool.tile([128, T], F32, tag="ps1")
                        if OPT["s1_style"] == "memset":
                            # zero psum, then one accumulating matmul per
                            # chunk over its whole band
                            getattr(nc, OPT["s1_memset_eng"]).memset(
                                ps[:], 0.0)
                            for ks in range(NS):
                                lo, hi = ranges[b][ks]
                                j = b * NS + ks
                                nc.tensor.matmul(
                                    ps[:, lo:hi + 1],
                                    x_chunk(j, mh),
                                    sel_sb[j][:],
                                    start=False, stop=(ks == NS - 1),
                                    skip_group_check=True)
                        else:
                            cov = -1
                            for ks in range(NS):
                                lo, hi = ranges[b][ks]
                                j = b * NS + ks
                                last = ks == NS - 1
                                ov_hi = min(cov, hi)
                                if lo <= ov_hi:  # overlap part: accumulate
                                    nc.tensor.matmul(
                                        ps[:, lo:ov_hi + 1],
                                        x_chunk(j, mh),
                                        sel_sb[j][:, 0:ov_hi - lo + 1],
                                        start=False,
                                        stop=last and hi <= cov,
                                        skip_group_check=True)
                                if hi > cov:     # fresh part: reset-write
                                    f_lo = max(lo, cov + 1)
                                    nc.tensor.matmul(
                                        ps[:, f_lo:hi + 1],
                                        x_chunk(j, mh),
                                        sel_sb[j][:, f_lo - lo:hi - lo + 1],
                                        start=True, stop=last,
                                        skip_group_check=True)
                                    cov = hi
                        _copy(nc, OPT["m_copy"], m, ps[:])
                        mb.append(m)
                    if b == B_LOC - 1:
                        # ote, transposed layout, straight from SBUF
                        for mh in range(NH):
                            odma_o.dma_start(
                                otT_d[mh * 128:(mh + 1) * 128, :],
                                m_all[mh][:])

                    # stage 2: rep rows of b = meanT.T @ fc_w.T + fc_b
                    rsb = opool.tile([128, 2 * H], BF16, tag="rsb")
                    for tq in range(2):
                        if OPT["s2_interleave"]:
                            # one stationary load per kh feeds both nh halves
                            pss = []
                            for _nh in range(2):
                                ps2i = p2pool.tile([128, NB2], F32,
                                                   tag="ps2")
                                pss.append(ps2i)
                            for kh in range(NH):
                                for nh in range(2):
                                    nsl = slice(nh * NB2, (nh + 1) * NB2)
                                    nc.tensor.matmul(
                                        pss[nh][:],
                                        mb[kh][:, tq * 128:(tq + 1) * 128],
                                        w_sb[kh][:, nsl],
                                        start=(kh == 0),
                                        stop=(not bias_mm and kh == NH - 1))
                            for nh in range(2):
                                nsl = slice(nh * NB2, (nh + 1) * NB2)
                                if bias_mm:
                                    nc.tensor.matmul(
                                        pss[nh][:], ones[:1, :],
                                        fcb_sb[:1, nsl],
                                        start=False, stop=True)
                                _copy(nc, OPT["rep_copy"],
                                      rsb[:, tq * H + nh * NB2:
                                          tq * H + (nh + 1) * NB2],
                                      pss[nh][:])
                            continue
                        for nh in range(2):
                            ps2 = p2pool.tile([128, NB2], F32, tag="ps2")
                            nsl = slice(nh * NB2, (nh + 1) * NB2)
                            for kh in range(NH):
                                nc.tensor.matmul(
                                    ps2[:],
                                    mb[kh][:, tq * 128:(tq + 1) * 128],
                                    w_sb[kh][:, nsl],
                                    start=(kh == 0),
                                    stop=(not bias_mm and kh == NH - 1))
                            if bias_mm:
                                nc.tensor.matmul(
                                    ps2[:], ones[:1, :], fcb_sb[:1, nsl],
                                    start=False, stop=True)
                            _copy(nc, OPT["rep_copy"],
                                  rsb[:, tq * H + nh * NB2:
                                      tq * H + (nh + 1) * NB2], ps2[:])
                    for tq in range(2):
                        r0 = (b * 2 + tq) * 128
                        odma_r.dma_start(rep_d[r0:r0 + 128, :],
                                         rsb[:, tq * H:(tq + 1) * H])

            if loop and reps > 1:
                u = OPT["unroll"]
                if reps % u:
                    u = 1
                with tc.For_i(0, reps // u, 1,
                              hint_engines=(mybir.EngineType.PE,)):
                    for _ in range(u):
                        emit_rep()
            else:
                for _ in range(reps):
                    emit_rep()

    nc.compile()
    return nc


def _host_prep(all_encoder_layers, token_map, fc_w, fc_b, layer_index):
    """Slice the chosen layer, build per-core input maps (bf16 I/O) and the
    per-(b_loc, s-chunk) active column bands (union over cores)."""
    layer = int(np.asarray(layer_index))
    x_full = np.asarray(all_encoder_layers)[layer]                # (B, S, H)
    tm = np.asarray(token_map).astype(np.int64)                   # (B, T+1)

    pos = np.arange(S)
    seg = np.empty((B, S), dtype=np.int64)
    for b in range(B):
        seg[b] = np.searchsorted(tm[b], pos, side="right") - 1
    valid = pos[None, :] < tm[:, -1:]
    seg = np.where(valid, np.clip(seg, 0, T - 1), T)              # (B, S)
    counts = (tm[:, 1:] - tm[:, :-1]).astype(np.float32)          # (B, T)
    inv = np.zeros((B, S), dtype=np.float32)
    bb = np.arange(B)[:, None]
    iv = seg < T
    inv[iv] = (np.float32(1.0) /
               counts[np.broadcast_to(bb, seg.shape)[iv], seg[iv]])

    # active column band per (local b, s-chunk): union over the 8 cores of
    # [seg at chunk start, seg at chunk end] (clipped to real segments)
    segc = np.clip(seg, 0, T - 1)
    ranges = []
    for b_loc in range(B_LOC):
        rows = [c * B_LOC + b_loc for c in range(N_CORES)]
        rb = []
        for ks in range(NS):
            lo = int(segc[rows, ks * 128].min())
            hi = int(segc[rows, ks * 128 + 127].max())
            rb.append((lo, hi))
        ranges.append(tuple(rb))
    ranges = tuple(ranges)

    fcwT = np.ascontiguousarray(
        np.asarray(fc_w, dtype=np.float32).T).astype(NP_BF16)
    fcb = np.asarray(fc_b, dtype=np.float32).reshape(1, H)

    x_bf = np.asarray(x_full, dtype=np.float32).astype(NP_BF16)
    in_maps = []
    for c in range(N_CORES):
        bs = slice(c * B_LOC, (c + 1) * B_LOC)
        # (B_LOC, S) -> (128, NJ) with column j = b*NS + chunk
        seg_t = seg[bs].reshape(NJ, 128).T.astype(np.float32)
        inv_t = inv[bs].reshape(NJ, 128).T
        aux = np.ascontiguousarray(
            np.concatenate([seg_t, inv_t], axis=1))          # (128, 2*NJ) f32
        bias_aux = np.ascontiguousarray(np.concatenate(
            [fcb, np.ones((1, 128), np.float32)], axis=1)).astype(NP_BF16)
        in_maps.append({
            "x": np.ascontiguousarray(x_bf[bs].reshape(NJ * 128, H)),
            "selaux": aux,
            "biasaux": bias_aux,
            "fcwT": fcwT,
        })
    return in_maps, ranges


class CachedRunner:
    """Jit/compile/load the bass program once; later calls are pure executes."""

    def __init__(self, nc, donate: bool = True):
        import jax
        from jax.sharding import Mesh, PartitionSpec
        from jax.experimental.shard_map import shard_map
        from concourse import bass2jax

        bass2jax.install_neuronx_cc_hook()
        self.nc = nc
        in_names, out_names, out_avals = [], [], []
        pname = nc.partition_id_tensor.name if nc.partition_id_tensor else None
        for alloc in nc.m.functions[0].allocations:
            if not isinstance(alloc, mybir.MemoryLocationSet):
                continue
            name = alloc.memorylocations[0].name
            if alloc.kind == "ExternalInput":
                if name != pname:
                    in_names.append(name)
            elif alloc.kind == "ExternalOutput":
                shape = tuple(alloc.tensor_shape)
                dtype = mybir.dt.np(alloc.dtype)
                out_names.append(name)
                out_avals.append(jax.core.ShapedArray(shape, dtype))
        self.in_names = list(in_names)
        self.out_names = out_names
        self.out_avals = out_avals
        n_params = len(in_names)
        n_outs = len(out_names)
        all_in_names = list(in_names) + list(out_names)
        if pname is not None:
            all_in_names.append(pname)
        donate_idx = tuple(range(n_params, n_params + n_outs)) if donate else ()

        def _body(*args):
            operands = list(args)
            if pname is not None:
                operands.append(bass2jax.partition_id_tensor())
            outs = bass2jax._bass_exec_p.bind(
                *operands,
                out_avals=tuple(out_avals),
                in_names=tuple(all_in_names),
                out_names=tuple(out_names),
                lowering_input_output_aliases=(),
                sim_require_finite=True,
                sim_require_nnan=True,
                nc=nc,
            )
            return tuple(outs)

        devices = jax.devices()[:N_CORES]
        mesh = Mesh(np.asarray(devices), ("core",))
        in_specs = (PartitionSpec("core"),) * (n_params + n_outs)
        out_specs = (PartitionSpec("core"),) * n_outs
        self.mesh = mesh
        self.sharding = jax.sharding.NamedSharding(mesh, PartitionSpec("core"))
        self.sharded = jax.jit(
            shard_map(_body, mesh=mesh, in_specs=in_specs,
                      out_specs=out_specs, check_rep=False),
            donate_argnums=donate_idx, keep_unused=True)
        self._dev_args = None

    def __call__(self, in_maps):
        concat_in = [
            np.concatenate([np.asarray(in_maps[c][n]) for c in range(N_CORES)], 0)
            for n in self.in_names]
        concat_zeros = [
            np.zeros((N_CORES * a.shape[0], *a.shape[1:]), a.dtype)
            for a in self.out_avals]
        out = self.sharded(*concat_in, *concat_zeros)
        return out  # list of jax arrays, concatenated over cores on axis 0

    def prepare(self, in_maps):
        """device_put all arguments once (requires donate=False runner)."""
        import jax
        concat_in = [
            np.concatenate([np.asarray(in_maps[c][n]) for c in range(N_CORES)], 0)
            for n in self.in_names]
        concat_zeros = [
            np.zeros((N_CORES * a.shape[0], *a.shape[1:]), a.dtype)
            for a in self.out_avals]
        self._dev_args = [jax.device_put(a, self.sharding)
                          for a in concat_in + concat_zeros]
        jax.block_until_ready(self._dev_args)

    def run_prepared(self):
        return self.sharded(*self._dev_args)


_RUNNER_CACHE: dict = {}


def get_runner(reps: int = 1, loop: bool = False, donate: bool = True,
               bias_mm: bool = False,
               ranges=DENSE_RANGES) -> CachedRunner:
    key = (reps, loop, donate, bias_mm, ranges)
    if key not in _RUNNER_CACHE:
        _RUNNER_CACHE[key] = CachedRunner(
            build_kernel(reps, loop, bias_mm=bias_mm, ranges=ranges), donate)
    return _RUNNER_CACHE[key]


def kernel(all_encoder_layers, input_mask, token_map, fc_w, fc_b, layer_index):
    in_maps, ranges = _host_prep(all_encoder_layers, token_map,
                                 fc_w, fc_b, layer_index)
    bias_mm = bool(np.any(np.asarray(fc_b)))
    runner = get_runner(1, bias_mm=bias_mm, ranges=ranges)
    out = runner(in_maps)
    idx = {n: i for i, n in enumerate(runner.out_names)}
    rep = np.asarray(out[idx["rep"]])                     # (8*1024, 768)
    otT = np.asarray(out[idx["otT"]])                     # (8*768, 1024)
    ote = (otT.reshape(N_CORES, H, B_LOC * T)
           .transpose(0, 2, 1).reshape(B * T, H))
    return rep.astype(np.float32), ote.astype(np.float32)


# revision 39
# speedup vs baseline: 1.0069x; 1.0069x over previous
"""Trainium2 Bass kernel for CharacterNet segment-mean + FC (segment_reduce).

Reference computation (per batch row b of 32):
  x = all_encoder_layers[layer_index][b]          # (512, 768)
  for t in 0..255: mean_t = mean(x[token_map[b,t]:token_map[b,t+1]])
  ote[b*256+t] = mean_t                           # (8192, 768) output 2
  rep = ote @ fc_w.T + fc_b                       # (8192, 768) output 1

Strategy: data-parallel over batch across 8 NeuronCores (4 rows/core).
The segment mean is a matmul with a one-hot-per-row selection matrix
SelT (512, 256), SelT[s, t] = (seg(s)==t) / count(seg(s)), built on
device from two tiny per-position index vectors with a single
tensor_scalar op per s-chunk.  Stage 1 computes meanT = x.T @ SelT
(H on partitions) so stage 2 (the FC) consumes it directly as the
stationary operand.

PE-cycle economy (the kernel is Tensor-engine bound):
 - All DRAM I/O is bf16 (inputs converted on host, outputs upcast on
   host); fp32 accumulation in PSUM keeps error ~4e-3 vs the 2e-2 gate.
 - Sparse stage 1: segments are contiguous s-ranges, so the 128-row
   s-chunk ks only touches output columns [lo_ks, hi_ks] (~100 of 256,
   union over the 8 SPMD cores, derived from token_map at build time).
   Matmuls cover only the active band: fresh columns start=True,
   columns shared with earlier chunks accumulate with start=False
   (seg jumps by <=1 per position, so bands are gap-free).
 - ote is written to DRAM transposed (oteT = meanT layout) straight
   from the stage-1 SBUF tiles - no PE transposes, no extra evictions;
   the host transposes for free during reassembly.
 - The K=1 bias matmuls are skipped when fc_b == 0.
"""

import numpy as np
import ml_dtypes

import concourse.bass as bass
import concourse.bacc as bacc
import concourse.mybir as mybir
import concourse.tile as tile
from concourse.bass_utils import run_bass_kernel_spmd  # noqa: F401 (contract)

N_CORES = 8
B, S, H, T = 32, 512, 768, 256
B_LOC = B // N_CORES          # 4 batch rows per core
NS = S // 128                 # 4 s-chunks per row
NJ = B_LOC * NS               # 16 (128,768) x chunks per core
NH = H // 128                 # 6 h-chunks
NB2 = 384                     # stage-2 N tile (two per 768)

F32 = mybir.dt.float32
BF16 = mybir.dt.bfloat16
NP_BF16 = ml_dtypes.bfloat16

# tunables: engine for each PSUM-evict copy class, DMA trigger engines,
# psum pool sizes, DMA emission order
OPT = {
    "m_copy": "vector",      # meanT psum->sbuf: vector | scalar
    "rep_copy": "scalar",    # stage2 psum->sbuf: vector | scalar
    "p1": 4, "p2": 4,
    # DMA trigger engines (s=sync, a=scalar/Act, g=gpsimd/Pool)
    "x_eng_map": "ssssssss",  # one char per x pair-DMA
    "w_eng": "s",
    "aux_eng": "s",
    "rep_eng_map": "aaaa",    # one char per b
    "ote_eng_map": "gggggg",  # one char per mh chunk
    "w_after": 3,            # emit fc_w DMAs after this many x pair-DMAs
    "unroll": 2,             # emit_rep per For_i iteration (loop mode)
    "dbl": True,             # bufs=2 on x/sel/m/w pools (pairs with unroll=2)
    "s1_style": "split",     # split (fresh/overlap matmuls) | memset
    "s1_memset_eng": "vector",
    "s2_interleave": True,   # reuse stage-2 stationary across both nh halves
}

# dense fallback ranges: every chunk covers all T columns
DENSE_RANGES = tuple(
    tuple((0, T - 1) for _ in range(NS)) for _ in range(B_LOC))


def _copy(nc, engine, dst, src_):
    if engine == "scalar":
        nc.scalar.copy(dst, src_)
    else:
        nc.vector.tensor_copy(dst, src_)


_ENG = {"s": "sync", "a": "scalar", "g": "gpsimd"}


def _dma(nc, eng_char, dst, src_):
    getattr(nc, _ENG[eng_char]).dma_start(dst, src_)


def build_kernel(reps: int = 1, loop: bool = False,
                 bias_mm: bool = False,
                 ranges=DENSE_RANGES) -> bass.Bass:
    nc = bacc.Bacc("TRN2", target_bir_lowering=False, debug=False,
                   num_devices=N_CORES)

    x_d = nc.dram_tensor("x", (NJ * 128, H), BF16, kind="ExternalInput")
    # packed aux: cols 0..15 = seg, 16..31 = inv  (128, 32)
    # (fp32: tensor_scalar is_equal requires fp32 scalar operands)
    aux_d = nc.dram_tensor("selaux", (128, 2 * NJ), F32, kind="ExternalInput")
    fcw_d = nc.dram_tensor("fcwT", (H, H), BF16, kind="ExternalInput")
    # packed bias row: [0:H]=fc_b, [H:H+128]=ones
    bias_d = nc.dram_tensor("biasaux", (1, H + 128), BF16, kind="ExternalInput")
    rep_d = nc.dram_tensor("rep", (B_LOC * T, H), BF16, kind="ExternalOutput")
    # ote in meanT orientation; host transposes during reassembly
    otT_d = nc.dram_tensor("otT", (H, B_LOC * T), BF16, kind="ExternalOutput")

    # paired-row-chunk view for big DMAs: [j0][p, q, h] = t[(2*j0+q)*128+p, h]
    x_v = x_d.rearrange("(a q p) h -> a p q h", q=2, p=128)

    dbl = 2 if OPT["dbl"] else 1
    with tile.TileContext(nc) as tc:
        with (
            tc.tile_pool(name="const", bufs=1) as cpool,
            tc.tile_pool(name="xp", bufs=dbl) as xpool,
            tc.tile_pool(name="selp", bufs=dbl) as selpool,
            tc.tile_pool(name="mp", bufs=dbl) as mpool,
            tc.tile_pool(name="wp", bufs=dbl) as wpool,
            tc.tile_pool(name="ob", bufs=2) as opool,
            tc.tile_pool(name="p1", bufs=OPT["p1"], space="PSUM") as p1pool,
            tc.tile_pool(name="p2", bufs=OPT["p2"], space="PSUM") as p2pool,
        ):
            # one-time constants
            iota_t = cpool.tile([128, T], BF16, tag="iota")
            nc.gpsimd.iota(iota_t[:], pattern=[[1, T]], base=0,
                           channel_multiplier=0,
                           allow_small_or_imprecise_dtypes=True)

            def emit_rep():
                aux_sb = cpool.tile([128, 2 * NJ], F32, tag="aux")
                bias_sb = cpool.tile([1, H + 128], BF16, tag="bias")
                _dma(nc, OPT["aux_eng"], aux_sb[:], aux_d[:])
                _dma(nc, OPT["aux_eng"], bias_sb[:], bias_d[:])
                fcb_sb = bias_sb[:1, 0:H]
                ones = bias_sb[:1, H:H + 128]

                w_sb, x2_sb = [], []

                def emit_w():
                    for k in range(NH):
                        w = wpool.tile([128, H], BF16, tag=f"w{k}")
                        _dma(nc, OPT["w_eng"], w[:],
                             fcw_d[k * 128:(k + 1) * 128, :])
                        w_sb.append(w)

                for j0 in range(NJ // 2):
                    if j0 == OPT["w_after"]:
                        emit_w()
                    x2 = xpool.tile([128, 2 * H], BF16, tag=f"x{j0}")
                    eng = OPT["x_eng_map"][j0 % len(OPT["x_eng_map"])]
                    if j0 == 0:
                        # first pair as two DMAs for an earlier PE start
                        for q in range(2):
                            _dma(nc, eng, x2[:, q * H:(q + 1) * H],
                                 x_d[q * 128:(q + 1) * 128, :])
                    else:
                        _dma(nc, eng,
                             x2[:].rearrange("p (q h) -> p q h", q=2),
                             x_v[j0])
                    x2_sb.append(x2)
                if OPT["w_after"] >= NJ // 2:
                    emit_w()

                def x_chunk(j, mh):
                    # (128,128) stationary slice of wp-token chunk j, h-chunk mh
                    q, j0 = j % 2, j // 2
                    o = q * H + mh * 128
                    return x2_sb[j0][:, o:o + 128]

                sel_sb = {}
                for b in range(B_LOC):
                    for ks in range(NS):
                        lo, hi = ranges[b][ks]
                        j = b * NS + ks
                        sel = selpool.tile([128, hi - lo + 1], BF16,
                                           tag=f"s{j}")
                        # Sel^T band: (s==seg member of segment t) / count
                        nc.vector.tensor_scalar(
                            sel[:], iota_t[:, lo:hi + 1],
                            aux_sb[:, j:j + 1], aux_sb[:, NJ + j:NJ + j + 1],
                            op0=mybir.AluOpType.is_equal,
                            op1=mybir.AluOpType.mult)
                        sel_sb[j] = sel

                # meanT rows for all 4 b side by side: one wide oteT DMA
                # per h-chunk (2KB per-partition lines)
                m_all = []
                for mh in range(NH):
                    m_row = mpool.tile([128, B_LOC * T], BF16, tag=f"m{mh}")
                    m_all.append(m_row)
                # rep rows for all (b, tq) side by side: one wide DMA per rep
                rsb = opool.tile([128, 2 * B_LOC * H], BF16, tag="rsb")
                for b in range(B_LOC):
                    # stage 1: meanT[b] (768, 256) = x[b].T @ SelT[b]
                    # split each chunk's band into already-covered columns
                    # (accumulate) and fresh columns (start=True)
                    mb = []
                    for mh in range(NH):
                        m = m_all[mh][:, b * T:(b + 1) * T]
                        ps = p1pool.tile([128, T], F32, tag="ps1")
                        if OPT["s1_style"] == "memset":
                            # zero psum, then one accumulating matmul per
                            # chunk over its whole band
                            getattr(nc, OPT["s1_memset_eng"]).memset(
                                ps[:], 0.0)
                            for ks in range(NS):
                                lo, hi = ranges[b][ks]
                                j = b * NS + ks
                                nc.tensor.matmul(
                                    ps[:, lo:hi + 1],
                                    x_chunk(j, mh),
                                    sel_sb[j][:],
                                    start=False, stop=(ks == NS - 1),
                                    skip_group_check=True)
                        else:
                            cov = -1
                            for ks in range(NS):
                                lo, hi = ranges[b][ks]
                                j = b * NS + ks
                                last = ks == NS - 1
                                ov_hi = min(cov, hi)
                                if lo <= ov_hi:  # overlap part: accumulate
                                    nc.tensor.matmul(
                                        ps[:, lo:ov_hi + 1],
                                        x_chunk(j, mh),
                                        sel_sb[j][:, 0:ov_hi - lo + 1],
                                        start=False,
                                        stop=last and hi <= cov,
                                        skip_group_check=True)
                                if hi > cov:     # fresh part: reset-write
                                    f_lo = max(lo, cov + 1)
                                    nc.tensor.matmul(
                                        ps[:, f_lo:hi + 1],
                                        x_chunk(j, mh),
                                        sel_sb[j][:, f_lo - lo:hi - lo + 1],
                                        start=True, stop=last,
                                        skip_group_check=True)
                                    cov = hi
                        _copy(nc, OPT["m_copy"], m, ps[:])
                        mb.append(m)
                    if b == B_LOC - 1:
                        # ote, transposed layout, straight from SBUF
                        for mh in range(NH):
                            _dma(nc, OPT["ote_eng_map"][mh],
                                 otT_d[mh * 128:(mh + 1) * 128, :],
                                 m_all[mh][:])

                    # stage 2: rep rows of b = meanT.T @ fc_w.T + fc_b
                    for tq in range(2):
                        rb = (2 * b + tq) * H
                        if OPT["s2_interleave"]:
                            # one stationary load per kh feeds both nh halves
                            pss = []
                            for _nh in range(2):
                                ps2i = p2pool.tile([128, NB2], F32,
                                                   tag="ps2")
                                pss.append(ps2i)
                            for kh in range(NH):
                                for nh in range(2):
                                    nsl = slice(nh * NB2, (nh + 1) * NB2)
                                    nc.tensor.matmul(
                                        pss[nh][:],
                                        mb[kh][:, tq * 128:(tq + 1) * 128],
                                        w_sb[kh][:, nsl],
                                        start=(kh == 0),
                                        stop=(not bias_mm and kh == NH - 1))
                            for nh in range(2):
                                nsl = slice(nh * NB2, (nh + 1) * NB2)
                                if bias_mm:
                                    nc.tensor.matmul(
                                        pss[nh][:], ones[:1, :],
                                        fcb_sb[:1, nsl],
                                        start=False, stop=True)
                                _copy(nc, OPT["rep_copy"],
                                      rsb[:, rb + nh * NB2:
                                          rb + (nh + 1) * NB2],
                                      pss[nh][:])
                            continue
                        for nh in range(2):
                            ps2 = p2pool.tile([128, NB2], F32, tag="ps2")
                            nsl = slice(nh * NB2, (nh + 1) * NB2)
                            for kh in range(NH):
                                nc.tensor.matmul(
                                    ps2[:],
                                    mb[kh][:, tq * 128:(tq + 1) * 128],
                                    w_sb[kh][:, nsl],
                                    start=(kh == 0),
                                    stop=(not bias_mm and kh == NH - 1))
                            if bias_mm:
                                nc.tensor.matmul(
                                    ps2[:], ones[:1, :], fcb_sb[:1, nsl],
                                    start=False, stop=True)
                            _copy(nc, OPT["rep_copy"],
                                  rsb[:, rb + nh * NB2:
                                      rb + (nh + 1) * NB2], ps2[:])
                    for tq in range(2):
                        r0 = (2 * b + tq) * 128
                        _dma(nc, OPT["rep_eng_map"][b],
                             rep_d[r0:r0 + 128, :],
                             rsb[:, (2 * b + tq) * H:(2 * b + tq + 1) * H])

            if loop and reps > 1:
                u = OPT["unroll"]
                if reps % u:
                    u = 1
                with tc.For_i(0, reps // u, 1,
                              hint_engines=(mybir.EngineType.PE,)):
                    for _ in range(u):
                        emit_rep()
            else:
                for _ in range(reps):
                    emit_rep()

    nc.compile()
    return nc


def _host_prep(all_encoder_layers, token_map, fc_w, fc_b, layer_index):
    """Slice the chosen layer, build per-core input maps (bf16 I/O) and the
    per-(b_loc, s-chunk) active column bands (union over cores)."""
    layer = int(np.asarray(layer_index))
    x_full = np.asarray(all_encoder_layers)[layer]                # (B, S, H)
    tm = np.asarray(token_map).astype(np.int64)                   # (B, T+1)

    pos = np.arange(S)
    seg = np.empty((B, S), dtype=np.int64)
    for b in range(B):
        seg[b] = np.searchsorted(tm[b], pos, side="right") - 1
    valid = pos[None, :] < tm[:, -1:]
    seg = np.where(valid, np.clip(seg, 0, T - 1), T)              # (B, S)
    counts = (tm[:, 1:] - tm[:, :-1]).astype(np.float32)          # (B, T)
    inv = np.zeros((B, S), dtype=np.float32)
    bb = np.arange(B)[:, None]
    iv = seg < T
    inv[iv] = (np.float32(1.0) /
               counts[np.broadcast_to(bb, seg.shape)[iv], seg[iv]])

    # active column band per (local b, s-chunk): union over the 8 cores of
    # [seg at chunk start, seg at chunk end] (clipped to real segments)
    segc = np.clip(seg, 0, T - 1)
    ranges = []
    for b_loc in range(B_LOC):
        rows = [c * B_LOC + b_loc for c in range(N_CORES)]
        rb = []
        for ks in range(NS):
            lo = int(segc[rows, ks * 128].min())
            hi = int(segc[rows, ks * 128 + 127].max())
            rb.append((lo, hi))
        ranges.append(tuple(rb))
    ranges = tuple(ranges)

    fcwT = np.ascontiguousarray(
        np.asarray(fc_w, dtype=np.float32).T).astype(NP_BF16)
    fcb = np.asarray(fc_b, dtype=np.float32).reshape(1, H)

    x_bf = np.asarray(x_full, dtype=np.float32).astype(NP_BF16)
    in_maps = []
    for c in range(N_CORES):
        bs = slice(c * B_LOC, (c + 1) * B_LOC)
        # (B_LOC, S) -> (128, NJ) with column j = b*NS + chunk
        seg_t = seg[bs].reshape(NJ, 128).T.astype(np.float32)
        inv_t = inv[bs].reshape(NJ, 128).T
        aux = np.ascontiguousarray(
            np.concatenate([seg_t, inv_t], axis=1))          # (128, 2*NJ) f32
        bias_aux = np.ascontiguousarray(np.concatenate(
            [fcb, np.ones((1, 128), np.float32)], axis=1)).astype(NP_BF16)
        in_maps.append({
            "x": np.ascontiguousarray(x_bf[bs].reshape(NJ * 128, H)),
            "selaux": aux,
            "biasaux": bias_aux,
            "fcwT": fcwT,
        })
    return in_maps, ranges


class CachedRunner:
    """Jit/compile/load the bass program once; later calls are pure executes."""

    def __init__(self, nc, donate: bool = True):
        import jax
        from jax.sharding import Mesh, PartitionSpec
        from jax.experimental.shard_map import shard_map
        from concourse import bass2jax

        bass2jax.install_neuronx_cc_hook()
        self.nc = nc
        in_names, out_names, out_avals = [], [], []
        pname = nc.partition_id_tensor.name if nc.partition_id_tensor else None
        for alloc in nc.m.functions[0].allocations:
            if not isinstance(alloc, mybir.MemoryLocationSet):
                continue
            name = alloc.memorylocations[0].name
            if alloc.kind == "ExternalInput":
                if name != pname:
                    in_names.append(name)
            elif alloc.kind == "ExternalOutput":
                shape = tuple(alloc.tensor_shape)
                dtype = mybir.dt.np(alloc.dtype)
                out_names.append(name)
                out_avals.append(jax.core.ShapedArray(shape, dtype))
        self.in_names = list(in_names)
        self.out_names = out_names
        self.out_avals = out_avals
        n_params = len(in_names)
        n_outs = len(out_names)
        all_in_names = list(in_names) + list(out_names)
        if pname is not None:
            all_in_names.append(pname)
        donate_idx = tuple(range(n_params, n_params + n_outs)) if donate else ()

        def _body(*args):
            operands = list(args)
            if pname is not None:
                operands.append(bass2jax.partition_id_tensor())
            outs = bass2jax._bass_exec_p.bind(
                *operands,
                out_avals=tuple(out_avals),
                in_names=tuple(all_in_names),
                out_names=tuple(out_names),
                lowering_input_output_aliases=(),
                sim_require_finite=True,
                sim_require_nnan=True,
                nc=nc,
            )
            return tuple(outs)

        devices = jax.devices()[:N_CORES]
        mesh = Mesh(np.asarray(devices), ("core",))
        in_specs = (PartitionSpec("core"),) * (n_params + n_outs)
        out_specs = (PartitionSpec("core"),) * n_outs
        self.mesh = mesh
        self.sharding = jax.sharding.NamedSharding(mesh, PartitionSpec("core"))
        self.sharded = jax.jit(
            shard_map(_body, mesh=mesh, in_specs=in_specs,
                      out_specs=out_specs, check_rep=False),
            donate_argnums=donate_idx, keep_unused=True)
        self._dev_args = None

    def __call__(self, in_maps):
        concat_in = [
            np.concatenate([np.asarray(in_maps[c][n]) for c in range(N_CORES)], 0)
            for n in self.in_names]
        concat_zeros = [
            np.zeros((N_CORES * a.shape[0], *a.shape[1:]), a.dtype)
            for a in self.out_avals]
        out = self.sharded(*concat_in, *concat_zeros)
        return out  # list of jax arrays, concatenated over cores on axis 0

    def prepare(self, in_maps):
        """device_put all arguments once (requires donate=False runner)."""
        import jax
        concat_in = [
            np.concatenate([np.asarray(in_maps[c][n]) for c in range(N_CORES)], 0)
            for n in self.in_names]
        concat_zeros = [
            np.zeros((N_CORES * a.shape[0], *a.shape[1:]), a.dtype)
            for a in self.out_avals]
        self._dev_args = [jax.device_put(a, self.sharding)
                          for a in concat_in + concat_zeros]
        jax.block_until_ready(self._dev_args)

    def run_prepared(self):
        return self.sharded(*self._dev_args)


_RUNNER_CACHE: dict = {}


def get_runner(reps: int = 1, loop: bool = False, donate: bool = True,
               bias_mm: bool = False,
               ranges=DENSE_RANGES) -> CachedRunner:
    key = (reps, loop, donate, bias_mm, ranges)
    if key not in _RUNNER_CACHE:
        _RUNNER_CACHE[key] = CachedRunner(
            build_kernel(reps, loop, bias_mm=bias_mm, ranges=ranges), donate)
    return _RUNNER_CACHE[key]


def kernel(all_encoder_layers, input_mask, token_map, fc_w, fc_b, layer_index):
    in_maps, ranges = _host_prep(all_encoder_layers, token_map,
                                 fc_w, fc_b, layer_index)
    bias_mm = bool(np.any(np.asarray(fc_b)))
    runner = get_runner(1, bias_mm=bias_mm, ranges=ranges)
    out = runner(in_maps)
    idx = {n: i for i, n in enumerate(runner.out_names)}
    rep = np.asarray(out[idx["rep"]])                     # (8*1024, 768)
    otT = np.asarray(out[idx["otT"]])                     # (8*768, 1024)
    ote = (otT.reshape(N_CORES, H, B_LOC * T)
           .transpose(0, 2, 1).reshape(B * T, H))
    return rep.astype(np.float32), ote.astype(np.float32)
